# revision 18
# baseline (speedup 1.0000x reference)
"""Trainium2 Bass kernel for nn_AntisymmetricLayer.

Computes, per token n (with z = x1-x2, s = x1+x2):
    out[n,k] = sum_d z[n,d] W[k,d]  +  sum_r (sum_d z[n,d] P[k,d,r]) * (sum_d s[n,d] Q[k,d,r])

Sharding: data-parallel over the batch dim (8 batches -> 8 NeuronCores),
weights replicated, no collectives.

Per-core pipeline (tokens N=16384, D=128, K=64, R=16), per 128-token tile:
  DMA(SWDGE): x1/x2 chunks DRAM f32 -> SBUF bf16 (cast in flight)
  GpSimd    : z = x1-x2, s = x1+x2 (bf16, natural [n,d] layout)
  DMA(xbar) : transpose z,s -> z^T,s^T [d,n] bf16 in SBUF
  PE        : A = z @ P2 [128,1024], B = s @ Q2, lin = z @ W^T (bf16 matmuls)
  ACT       : stage B PSUM -> SBUF bf16; copy lin into prod slot 17
  DVE       : prod[:, k, 0:16] = A*B; out = reduce over 17 (16 prods + lin)
  DMA       : per-tile stores
"""

import numpy as np
import ml_dtypes

import concourse.bass as bass
import concourse.mybir as mybir
import concourse.tile as tile
from concourse import bacc
from concourse.bass import ts
from concourse.bass_utils import run_bass_kernel_spmd

F32 = mybir.dt.float32
BF16 = mybir.dt.bfloat16

D = 128
K = 64
R = 16
KR = K * R  # 1024
CONST_W = 2 * KR + K  # p2|q2|wt packed
N_CORES = 8
TILE = 128          # tokens per tile (partition dim)
CHUNK_TILES = 4     # tiles loaded per input DMA (512 tokens)


def build_bass(n_tokens: int = 16384):
    """Build the per-core Bass program. All cores run the same program on
    their own batch shard."""
    assert n_tokens % (TILE * CHUNK_TILES) == 0
    n_tiles = n_tokens // TILE
    n_chunks = n_tiles // CHUNK_TILES

    nc = bacc.Bacc(None, target_bir_lowering=False)

    x1 = nc.declare_dram_parameter("x1", [n_tokens, D], F32, isOutput=False)
    x2 = nc.declare_dram_parameter("x2", [n_tokens, D], F32, isOutput=False)
    cw = nc.declare_dram_parameter("cw", [D, CONST_W], BF16, isOutput=False)
    out = nc.declare_dram_parameter("out", [n_tokens, K], F32, isOutput=True)

    with tile.TileContext(nc) as tc:
        with (
            tc.tile_pool(name="const", bufs=1) as cpool,
            tc.tile_pool(name="xin", bufs=3) as xpool,
            tc.tile_pool(name="zs", bufs=4) as zpool,
            tc.tile_pool(name="zst", bufs=4) as ztpool,
            tc.tile_pool(name="prods", bufs=4) as ppool,
            tc.tile_pool(name="outs", bufs=4) as opool,
            tc.tile_pool(name="plin", bufs=2, space="PSUM") as plin_pool,
            tc.tile_pool(name="pab", bufs=3, space="PSUM") as pab_pool,
        ):
            # --- constants, loaded once (single DMA) ----------------------
            cws = cpool.tile([D, CONST_W], BF16)
            nc.sync.dma_start(cws[:], cw[:])
            p2s = cws[:, 0:KR]
            q2s = cws[:, KR : 2 * KR]
            wts = cws[:, 2 * KR : 2 * KR + K]

            x1v = x1.rearrange("(c a p) d -> c p a d", p=TILE, a=CHUNK_TILES)
            x2v = x2.rearrange("(c a p) d -> c p a d", p=TILE, a=CHUNK_TILES)

            # 1-tile software skew: tile i's front-end (z/s + transposes) is
            # emitted before tile i-1's matmuls/epilogue
            prev = None

            def do_tail(zt, st, i):
                # PE: main matmuls (bf16, f32 accumulate)
                a0 = pab_pool.tile([TILE, 512], F32, name=f"a0_{i}", tag="A")
                a1 = pab_pool.tile([TILE, 512], F32, name=f"a1_{i}", tag="A")
                b0 = pab_pool.tile([TILE, 512], F32, name=f"b0_{i}", tag="B")
                b1 = pab_pool.tile([TILE, 512], F32, name=f"b1_{i}", tag="B")
                lin = plin_pool.tile([TILE, K], F32, name=f"lin{i}", tag="lin")
                nc.tensor.matmul(a0[:], zt[:], p2s[:, 0:512], start=True, stop=True)
                nc.tensor.matmul(a1[:], zt[:], p2s[:, 512:1024], start=True, stop=True)
                nc.tensor.matmul(lin[:], zt[:], wts, start=True, stop=True)
                nc.tensor.matmul(b0[:], st[:], q2s[:, 0:512], start=True, stop=True)
                nc.tensor.matmul(b1[:], st[:], q2s[:, 512:1024], start=True, stop=True)

                # ACT: stage B in SBUF bf16 (DVE tensor_tensor reads at most
                # one PSUM operand); copy lin into prod slot 17
                b0s = ppool.tile([TILE, 512], BF16, name=f"b0s{i}", tag="b0s")
                nc.scalar.copy(b0s[:], b0[:])
                b1s = ppool.tile([TILE, 512], BF16, name=f"b1s{i}", tag="b1s")
                nc.scalar.copy(b1s[:], b1[:])

                # prod layout [p, k, 17]: slots 0:16 products, slot 16 lin
                prod = ppool.tile([TILE, K * 17], BF16, name=f"prod{i}", tag="prod")
                pv = prod.rearrange("p (k r) -> p k r", r=17)
                nc.scalar.copy(pv[:, :, 16:17], lin.rearrange("p (k o) -> p k o", o=1))

                # DVE: products then a single 17-wide segmented reduce
                nc.vector.tensor_mul(
                    pv[:, 0:32, 0:16],
                    a0.rearrange("p (k r) -> p k r", r=R),
                    b0s.rearrange("p (k r) -> p k r", r=R),
                )
                nc.vector.tensor_mul(
                    pv[:, 32:64, 0:16],
                    a1.rearrange("p (k r) -> p k r", r=R),
                    b1s.rearrange("p (k r) -> p k r", r=R),
                )
                fin = opool.tile([TILE, K], F32, name=f"fin{i}", tag="fin")
                nc.vector.reduce_sum(fin[:], pv, axis=mybir.AxisListType.X)
                nc.sync.dma_start(out[ts(i, TILE), :], fin[:])

            for c in range(n_chunks):
                # SWDGE DMA casts f32 DRAM -> bf16 SBUF in flight (full f32
                # read traffic from HBM, no compute-engine cost)
                x1c = xpool.tile([TILE, CHUNK_TILES, D], BF16, name=f"x1c{c}", tag="x1c")
                nc.gpsimd.dma_start(x1c[:], x1v[c])
                x2c = xpool.tile([TILE, CHUNK_TILES, D], BF16, name=f"x2c{c}", tag="x2c")
                nc.gpsimd.dma_start(x2c[:], x2v[c])

                for a in range(CHUNK_TILES):
                    i = c * CHUNK_TILES + a
                    # GpSimd: z/s in natural layout (SBUF bf16; GpSimd cannot
                    # touch PSUM but these are pure SBUF)
                    zn = zpool.tile([TILE, D], BF16, name=f"zn{i}", tag="zn")
                    nc.gpsimd.tensor_sub(zn[:], x1c[:, a, :], x2c[:, a, :])
                    sn = zpool.tile([TILE, D], BF16, name=f"sn{i}", tag="sn")
                    nc.gpsimd.tensor_add(sn[:], x1c[:, a, :], x2c[:, a, :])

                    # xbar DMA transpose (bf16 SBUF->SBUF on DMA engines):
                    # z^T, s^T [d, n]
                    zt = ztpool.tile([D, TILE], BF16, name=f"zt{i}", tag="zt")
                    nc.sync.dma_start(zt[:], zn[:], transpose=True)
                    st = ztpool.tile([D, TILE], BF16, name=f"st{i}", tag="st")
                    nc.sync.dma_start(st[:], sn[:], transpose=True)

                    if prev is not None:
                        do_tail(*prev)
                    prev = (zt, st, i)

            do_tail(*prev)

    nc.finalize()
    return nc


def _shard_and_pack(x1, x2, W_lin, P, Q):
    """Host-side: batch-shard x1/x2, repack weights (layout + bf16 cast)."""
    p2 = P.transpose(1, 0, 2).reshape(D, KR)
    q2 = Q.transpose(1, 0, 2).reshape(D, KR)
    wt = np.ascontiguousarray(W_lin.T)
    cwv = np.concatenate([p2, q2, wt], axis=1).astype(ml_dtypes.bfloat16)
    assert cwv.shape == (D, CONST_W)

    in_maps = []
    for b in range(N_CORES):
        in_maps.append(
            {
                "x1": np.ascontiguousarray(x1[b]),
                "x2": np.ascontiguousarray(x2[b]),
                "cw": cwv,
            }
        )
    return in_maps


def kernel(x1, x2, W_lin, P, Q):
    assert x1.shape == (N_CORES, 16384, D) and x2.shape == x1.shape
    nc = build_bass(16384)
    in_maps = _shard_and_pack(x1, x2, W_lin, P, Q)
    res = run_bass_kernel_spmd(nc, in_maps, core_ids=list(range(N_CORES)))
    out = np.stack([res.results[b]["out"] for b in range(N_CORES)], axis=0)
    return out.astype(np.float32)


# revision 19
# speedup vs baseline: 1.7257x; 1.7257x over previous
"""Trainium2 Bass kernel for nn_AntisymmetricLayer.

Computes, per token n (with z = x1-x2, s = x1+x2):
    out[n,k] = sum_d z[n,d] W[k,d]  +  sum_r (sum_d z[n,d] P[k,d,r]) * (sum_d s[n,d] Q[k,d,r])

Sharding: data-parallel over the batch dim (8 batches -> 8 NeuronCores),
weights replicated, no collectives.

Per-core pipeline (tokens N=16384, D=128, K=64, R=16), per 128-token tile:
  DMA(SWDGE): x1/x2 chunks DRAM f32 -> SBUF bf16 (cast in flight)
  PE        : z^T/s^T via +/-identity matmuls accumulating in one PSUM bank
              (sequential accumulation groups); A = z @ P2 [128,1024],
              B = s @ Q2, lin = z @ W^T (bf16 matmuls, f32 PSUM)
  ACT       : one copy z^T|s^T PSUM -> SBUF bf16; stage B in SBUF bf16;
              copy lin into prod slot 17
  DVE       : prod[:, k, 0:16] = A*B; out = reduce over 17 (16 prods + lin)
  DMA       : per-tile stores
"""

import numpy as np
import ml_dtypes

import concourse.bass as bass
import concourse.mybir as mybir
import concourse.tile as tile
from concourse import bacc
from concourse.bass import ts
from concourse.bass_utils import run_bass_kernel_spmd

F32 = mybir.dt.float32
BF16 = mybir.dt.bfloat16

D = 128
K = 64
R = 16
KR = K * R  # 1024
CONST_W = 2 * KR + K + 2 * 128  # p2|q2|wt|+I|-I packed
N_CORES = 8
TILE = 128          # tokens per tile (partition dim)
CHUNK_TILES = 4     # tiles loaded per input DMA (512 tokens)


def build_bass(n_tokens: int = 16384):
    """Build the per-core Bass program. All cores run the same program on
    their own batch shard."""
    assert n_tokens % (TILE * CHUNK_TILES) == 0
    n_tiles = n_tokens // TILE
    n_chunks = n_tiles // CHUNK_TILES

    nc = bacc.Bacc(None, target_bir_lowering=False)

    x1 = nc.declare_dram_parameter("x1", [n_tokens, D], F32, isOutput=False)
    x2 = nc.declare_dram_parameter("x2", [n_tokens, D], F32, isOutput=False)
    cw = nc.declare_dram_parameter("cw", [D, CONST_W], BF16, isOutput=False)
    out = nc.declare_dram_parameter("out", [n_tokens, K], F32, isOutput=True)

    with tile.TileContext(nc) as tc:
        with (
            tc.tile_pool(name="const", bufs=1) as cpool,
            tc.tile_pool(name="xin", bufs=3) as xpool,
            tc.tile_pool(name="zs", bufs=4) as zpool,
            tc.tile_pool(name="prods", bufs=4) as ppool,
            tc.tile_pool(name="outs", bufs=4) as opool,
            tc.tile_pool(name="ptr", bufs=2, space="PSUM") as ptr_pool,
            tc.tile_pool(name="pab", bufs=3, space="PSUM") as pab_pool,
        ):
            # --- constants, loaded once (single DMA) ----------------------
            cws = cpool.tile([D, CONST_W], BF16)
            nc.sync.dma_start(cws[:], cw[:])
            p2s = cws[:, 0:KR]
            q2s = cws[:, KR : 2 * KR]
            wts = cws[:, 2 * KR : 2 * KR + K]
            ident = cws[:, 2 * KR + K : 2 * KR + K + D]
            identn = cws[:, 2 * KR + K + D : 2 * KR + K + 2 * D]

            x1v = x1.rearrange("(c a p) d -> c p a d", p=TILE, a=CHUNK_TILES)
            x2v = x2.rearrange("(c a p) d -> c p a d", p=TILE, a=CHUNK_TILES)

            # 1-tile software skew: tile i's transposes are emitted before
            # tile i-1's main matmuls/epilogue
            prev = None

            def do_tail(pzs, lin, i):
                # ACT: single copy of z^T|s^T (PSUM f32 -> SBUF bf16)
                zst = zpool.tile([D, 2 * TILE], BF16, name=f"zst{i}", tag="zst")
                nc.scalar.copy(zst[:], pzs[:])
                zt = zst[:, 0:TILE]
                st = zst[:, TILE : 2 * TILE]

                # PE: main matmuls (bf16, f32 accumulate)
                a0 = pab_pool.tile([TILE, 512], F32, name=f"a0_{i}", tag="A")
                a1 = pab_pool.tile([TILE, 512], F32, name=f"a1_{i}", tag="A")
                b0 = pab_pool.tile([TILE, 512], F32, name=f"b0_{i}", tag="B")
                b1 = pab_pool.tile([TILE, 512], F32, name=f"b1_{i}", tag="B")
                nc.tensor.matmul(a0[:], zt, p2s[:, 0:512], start=True, stop=True)
                nc.tensor.matmul(a1[:], zt, p2s[:, 512:1024], start=True, stop=True)
                nc.tensor.matmul(lin, zt, wts, start=True, stop=True)
                nc.tensor.matmul(b0[:], st, q2s[:, 0:512], start=True, stop=True)
                nc.tensor.matmul(b1[:], st, q2s[:, 512:1024], start=True, stop=True)

                # ACT: stage B in SBUF bf16 (DVE tensor_tensor reads at most
                # one PSUM operand); copy lin into prod slot 17
                b0s = ppool.tile([TILE, 512], BF16, name=f"b0s{i}", tag="b0s")
                nc.scalar.copy(b0s[:], b0[:])
                b1s = ppool.tile([TILE, 512], BF16, name=f"b1s{i}", tag="b1s")
                nc.scalar.copy(b1s[:], b1[:])

                # prod layout [p, k, 17]: slots 0:16 products, slot 16 lin
                prod = ppool.tile([TILE, K * 17], BF16, name=f"prod{i}", tag="prod")
                pv = prod.rearrange("p (k r) -> p k r", r=17)
                nc.scalar.copy(
                    pv[:, :, 16:17],
                    lin.rearrange("p (k o) -> p k o", o=1),
                )

                # DVE: products then a single 17-wide segmented reduce
                nc.vector.tensor_mul(
                    pv[:, 0:32, 0:16],
                    a0.rearrange("p (k r) -> p k r", r=R),
                    b0s.rearrange("p (k r) -> p k r", r=R),
                )
                nc.vector.tensor_mul(
                    pv[:, 32:64, 0:16],
                    a1.rearrange("p (k r) -> p k r", r=R),
                    b1s.rearrange("p (k r) -> p k r", r=R),
                )
                fin = opool.tile([TILE, K], F32, name=f"fin{i}", tag="fin")
                nc.vector.reduce_sum(fin[:], pv, axis=mybir.AxisListType.X)
                nc.sync.dma_start(out[ts(i, TILE), :], fin[:])

            for c in range(n_chunks):
                # SWDGE DMA casts f32 DRAM -> bf16 SBUF in flight (full f32
                # read traffic from HBM, no compute-engine cost)
                x1c = xpool.tile([TILE, CHUNK_TILES, D], BF16, name=f"x1c{c}", tag="x1c")
                nc.gpsimd.dma_start(x1c[:], x1v[c])
                x2c = xpool.tile([TILE, CHUNK_TILES, D], BF16, name=f"x2c{c}", tag="x2c")
                nc.gpsimd.dma_start(x2c[:], x2v[c])

                for a in range(CHUNK_TILES):
                    i = c * CHUNK_TILES + a
                    # PE transposes via regular matmuls with +/-I moving:
                    # out = lhsT.T @ (+/-I). pz/ps/lin share ONE PSUM bank;
                    # accumulation groups run sequentially (pz then ps, lin
                    # later) which is legal within a bank.
                    combo = ptr_pool.tile([D, 512], F32, name=f"combo{i}", tag="trsp")
                    pz = combo[:, 0:TILE]
                    ps = combo[:, TILE : 2 * TILE]
                    lin = combo[:, 2 * TILE : 2 * TILE + K]
                    x1b = x1c[:, a, :]
                    x2b = x2c[:, a, :]
                    nc.tensor.matmul(pz, x1b, ident, start=True, stop=False)
                    nc.tensor.matmul(pz, x2b, identn, start=False, stop=True)
                    nc.tensor.matmul(ps, x1b, ident, start=True, stop=False)
                    nc.tensor.matmul(ps, x2b, ident, start=False, stop=True)

                    if prev is not None:
                        do_tail(*prev)
                    prev = (combo[:, 0 : 2 * TILE], lin, i)

            do_tail(*prev)

    nc.finalize()
    return nc


def _shard_and_pack(x1, x2, W_lin, P, Q):
    """Host-side: batch-shard x1/x2, repack weights (layout + bf16 cast)."""
    p2 = P.transpose(1, 0, 2).reshape(D, KR)
    q2 = Q.transpose(1, 0, 2).reshape(D, KR)
    wt = np.ascontiguousarray(W_lin.T)
    idp = np.eye(D, dtype=np.float32)
    cwv = np.concatenate([p2, q2, wt, idp, -idp], axis=1).astype(ml_dtypes.bfloat16)
    assert cwv.shape == (D, CONST_W)

    in_maps = []
    for b in range(N_CORES):
        in_maps.append(
            {
                "x1": np.ascontiguousarray(x1[b]),
                "x2": np.ascontiguousarray(x2[b]),
                "cw": cwv,
            }
        )
    return in_maps


def kernel(x1, x2, W_lin, P, Q):
    assert x1.shape == (N_CORES, 16384, D) and x2.shape == x1.shape
    nc = build_bass(16384)
    in_maps = _shard_and_pack(x1, x2, W_lin, P, Q)
    res = run_bass_kernel_spmd(nc, in_maps, core_ids=list(range(N_CORES)))
    out = np.stack([res.results[b]["out"] for b in range(N_CORES)], axis=0)
    return out.astype(np.float32)


# revision 20
# speedup vs baseline: 3.2328x; 1.8734x over previous
"""Trainium2 Bass kernel for nn_AntisymmetricLayer.

Computes, per token n (with z = x1-x2, s = x1+x2):
    out[n,k] = sum_d z[n,d] W[k,d]  +  sum_r (sum_d z[n,d] P[k,d,r]) * (sum_d s[n,d] Q[k,d,r])

Sharding: data-parallel over the batch dim (8 batches -> 8 NeuronCores),
weights replicated, no collectives.

Per-core program (tokens N=16384, D=128, K=64, R=16):
  loop over 128-token tiles:
    PE   : transpose x1/x2 tiles with +/-identity accumulating into PSUM
           -> z^T, s^T (f32); then bf16 matmuls A = z @ P2 [128,1024],
           B = s @ Q2 [128,1024], lin = z @ W^T [128,64]
    ACT  : copy z^T/s^T PSUM -> SBUF with f32->bf16 cast
    DVE  : prod = A*B (PSUM x PSUM -> SBUF bf16), segmented reduce over
           r=16, add lin -> out tile [128,64] f32
    DMA  : chunked loads (512 tokens per dma), per-tile stores
"""

import numpy as np
import ml_dtypes

import concourse.bass as bass
import concourse.mybir as mybir
import concourse.tile as tile
from concourse import bacc
from concourse.bass import ts
from concourse.bass_utils import run_bass_kernel_spmd

F32 = mybir.dt.float32
BF16 = mybir.dt.bfloat16

D = 128
K = 64
R = 16
KR = K * R  # 1024
CONST_W = 2 * KR + K + 2 * 128  # p2|q2|wt|+I|-I packed
N_CORES = 8
TILE = 128          # tokens per tile (partition dim)
CHUNK_TILES = 4     # tiles loaded per input DMA (512 tokens)


def build_bass(n_tokens: int = 16384):
    """Build the per-core Bass program. All cores run the same program on
    their own batch shard."""
    assert n_tokens % (TILE * CHUNK_TILES) == 0
    n_tiles = n_tokens // TILE
    n_chunks = n_tiles // CHUNK_TILES

    nc = bacc.Bacc(None, target_bir_lowering=False)

    x1 = nc.declare_dram_parameter("x1", [n_tokens, D], F32, isOutput=False)
    x2 = nc.declare_dram_parameter("x2", [n_tokens, D], F32, isOutput=False)
    # all small constants packed into one tensor -> one load, one wait sem:
    # [p2 | q2 | wt | +I | -I] along the free dim
    cw = nc.declare_dram_parameter("cw", [D, CONST_W], BF16, isOutput=False)
    out = nc.declare_dram_parameter("out", [n_tokens, K], F32, isOutput=True)

    with tile.TileContext(nc) as tc:
        with (
            tc.tile_pool(name="const", bufs=1) as cpool,
            tc.tile_pool(name="xin", bufs=2) as xpool,
            tc.tile_pool(name="zs", bufs=3) as zpool,
            tc.tile_pool(name="prods", bufs=3) as ppool,
            tc.tile_pool(name="outs", bufs=3) as opool,
            tc.tile_pool(name="ptr", bufs=2, space="PSUM") as ptr_pool,
            tc.tile_pool(name="pab", bufs=2, space="PSUM") as pab_pool,
        ):
            # --- constants, loaded once (single DMA) ----------------------
            cws = cpool.tile([D, CONST_W], BF16)
            nc.sync.dma_start(cws[:], cw[:])
            p2s = cws[:, 0:KR]
            q2s = cws[:, KR : 2 * KR]
            wts = cws[:, 2 * KR : 2 * KR + K]
            ident = cws[:, 2 * KR + K : 2 * KR + K + D]
            identn = cws[:, 2 * KR + K + D : 2 * KR + K + 2 * D]

            x1v = x1.rearrange("(c a p) d -> c p a d", p=TILE, a=CHUNK_TILES)
            x2v = x2.rearrange("(c a p) d -> c p a d", p=TILE, a=CHUNK_TILES)

            # state carried across loop iterations for the 1-tile software
            # skew: tile i's transposes are emitted before tile i-1's matmuls
            prev = None  # (pz, ps, tile_idx)

            def do_tail(pz, ps, lin, i):
                # ACT: PSUM f32 -> SBUF bf16
                zt = zpool.tile([D, TILE], BF16, name=f"zt{i}", tag="zt")
                nc.scalar.copy(zt[:], pz[:])
                st = zpool.tile([D, TILE], BF16, name=f"st{i}", tag="st")
                nc.scalar.copy(st[:], ps[:])

                # PE: main matmuls (bf16, f32 accumulate)
                a0 = pab_pool.tile([TILE, 512], F32, name=f"a0_{i}", tag="A")
                a1 = pab_pool.tile([TILE, 512], F32, name=f"a1_{i}", tag="A")
                b0 = pab_pool.tile([TILE, 512], F32, name=f"b0_{i}", tag="B")
                b1 = pab_pool.tile([TILE, 512], F32, name=f"b1_{i}", tag="B")
                nc.tensor.matmul(a0[:], zt[:], p2s[:, 0:512], start=True, stop=True)
                nc.tensor.matmul(a1[:], zt[:], p2s[:, 512:1024], start=True, stop=True)
                nc.tensor.matmul(lin[:], zt[:], wts, start=True, stop=True)
                nc.tensor.matmul(b0[:], st[:], q2s[:, 0:512], start=True, stop=True)
                nc.tensor.matmul(b1[:], st[:], q2s[:, 512:1024], start=True, stop=True)

                # ACT: stage B in SBUF (bf16) -- DVE tensor_tensor may read at
                # most one PSUM operand
                b0s = ppool.tile([TILE, 512], BF16, name=f"b0s{i}", tag="b0s")
                nc.scalar.copy(b0s[:], b0[:])
                b1s = ppool.tile([TILE, 512], BF16, name=f"b1s{i}", tag="b1s")
                nc.scalar.copy(b1s[:], b1[:])

                # DVE: prod = A*B -> SBUF bf16 (one PSUM + one SBUF operand)
                prod = ppool.tile([TILE, KR], BF16, name=f"prod{i}", tag="prod")
                nc.vector.tensor_mul(prod[:, 0:512], a0[:], b0s[:])
                nc.vector.tensor_mul(prod[:, 512:1024], a1[:], b1s[:])

                # DVE: segmented reduce over r (innermost 16)
                red = opool.tile([TILE, K], F32, name=f"red{i}", tag="red")
                nc.vector.reduce_sum(
                    red[:],
                    prod.rearrange("p (k r) -> p k r", r=R),
                    axis=mybir.AxisListType.X,
                )
                # DVE: add linear term
                fin = opool.tile([TILE, K], F32, name=f"fin{i}", tag="fin")
                nc.vector.tensor_add(fin[:], red[:], lin[:])
                nc.sync.dma_start(out[ts(i, TILE), :], fin[:])

            for c in range(n_chunks):
                # SWDGE DMA casts f32 DRAM -> bf16 SBUF in flight (full f32
                # read traffic from HBM, no compute-engine cost)
                x1c = xpool.tile([TILE, CHUNK_TILES, D], BF16, name=f"x1c{c}", tag="x1c")
                nc.gpsimd.dma_start(x1c[:], x1v[c])
                x2c = xpool.tile([TILE, CHUNK_TILES, D], BF16, name=f"x2c{c}", tag="x2c")
                nc.gpsimd.dma_start(x2c[:], x2v[c])

                for a in range(CHUNK_TILES):
                    i = c * CHUNK_TILES + a
                    # PE: transpose via regular matmul with +/-I as the
                    # moving operand: out = lhsT.T @ I = lhsT^T.
                    # combo spans 2 PSUM banks: bank0 holds pz (+ lin later),
                    # bank1 holds ps -- the two accumulation groups interleave
                    # but target different banks.
                    # pz = x1^T - x2^T ; ps = x1^T + x2^T   (f32, PSUM)
                    combo = ptr_pool.tile([D, 1024], F32, name=f"combo{i}", tag="trsp")
                    pz = combo[:, 0:TILE]
                    lin = combo[:, 256 : 256 + K]
                    ps = combo[:, 512 : 512 + TILE]
                    x1b = x1c[:, a, :]
                    x2b = x2c[:, a, :]
                    nc.tensor.matmul(pz, x1b, ident, start=True, stop=False)
                    nc.tensor.matmul(ps, x1b, ident, start=True, stop=False)
                    nc.tensor.matmul(pz, x2b, identn, start=False, stop=True)
                    nc.tensor.matmul(ps, x2b, ident, start=False, stop=True)

                    if prev is not None:
                        do_tail(*prev)
                    prev = (pz, ps, lin, i)

            do_tail(*prev)

    nc.finalize()
    return nc


def _shard_and_pack(x1, x2, W_lin, P, Q):
    """Host-side: batch-shard x1/x2, repack weights (layout + bf16 cast)."""
    p2 = P.transpose(1, 0, 2).reshape(D, KR)
    q2 = Q.transpose(1, 0, 2).reshape(D, KR)
    wt = np.ascontiguousarray(W_lin.T)
    idp = np.eye(D, dtype=np.float32)
    cw = np.concatenate([p2, q2, wt, idp, -idp], axis=1).astype(ml_dtypes.bfloat16)
    assert cw.shape == (D, CONST_W)

    in_maps = []
    for b in range(N_CORES):
        in_maps.append(
            {
                "x1": np.ascontiguousarray(x1[b]),
                "x2": np.ascontiguousarray(x2[b]),
                "cw": cw,
            }
        )
    return in_maps


def kernel(x1, x2, W_lin, P, Q):
    assert x1.shape == (N_CORES, 16384, D) and x2.shape == x1.shape
    nc = build_bass(16384)
    in_maps = _shard_and_pack(x1, x2, W_lin, P, Q)
    res = run_bass_kernel_spmd(nc, in_maps, core_ids=list(range(N_CORES)))
    out = np.stack([res.results[b]["out"] for b in range(N_CORES)], axis=0)
    return out.astype(np.float32)


# revision 21
# speedup vs baseline: 3.2478x; 1.0047x over previous
"""Trainium2 Bass kernel for nn_AntisymmetricLayer — v4 (PE-side reduction).

Same math as kernel.py, but the r-reduction and the lin add run on the
TensorEngine via accumulating matmuls against a 0/1 selection matrix, so the
VectorEngine does ONLY the elementwise products.

Layout trick: computation runs transposed. Per 512-token block:
  GpSimd   : z = x1-x2, s = x1+x2 on whole block [128, 512] bf16
  DMA xbar : transpose -> z^T, s^T [d, n-block] bf16
  PE       : A^T_c = P2_c^T @ z^T  [128kr, 512n] (8 chunks of kr), B^T_c same
             outT = W^T-matmul (lin, start) + sum_c sel_c^T @ prod_c (accum)
  ACT      : stage B^T_c PSUM -> SBUF bf16; evacuate outT -> SBUF
  DVE      : prod_c = A^T_c * B^T_c  (one PSUM + one SBUF operand)
  out in DRAM is [K, n_tokens]; host transposes during unshard.

sel_c[p, k] = 1 iff k == c*8 + p//16  (sums groups of 16 kr-partitions)
"""

import numpy as np
import ml_dtypes

import concourse.bass as bass
import concourse.mybir as mybir
import concourse.tile as tile
from concourse import bacc
from concourse.bass import ts
from concourse.bass_utils import run_bass_kernel_spmd

F32 = mybir.dt.float32
BF16 = mybir.dt.bfloat16

D = 128
K = 64
R = 16
KR = K * R  # 1024
NCHUNK = KR // 128  # 8 kr-chunks of 128
SELW = NCHUNK * K   # 512
CONST_W = 2 * KR + K + SELW + 2 * 128  # p2|q2|wt|sel|+I|-I packed
N_CORES = 8
OUT_T = True  # DRAM output is [K, n]; host transposes
TILE = 128
CHUNK_TILES = 4     # tokens per block = 512
BLK = TILE * CHUNK_TILES


def build_bass(n_tokens: int = 16384):
    assert n_tokens % BLK == 0
    n_blocks = n_tokens // BLK

    nc = bacc.Bacc(None, target_bir_lowering=False)

    x1 = nc.declare_dram_parameter("x1", [n_tokens, D], F32, isOutput=False)
    x2 = nc.declare_dram_parameter("x2", [n_tokens, D], F32, isOutput=False)
    cw = nc.declare_dram_parameter("cw", [D, CONST_W], BF16, isOutput=False)
    # output stored transposed [K, n]; host transposes after gather
    out = nc.declare_dram_parameter("out", [K, n_tokens], F32, isOutput=True)

    with tile.TileContext(nc) as tc:
        with (
            tc.tile_pool(name="const", bufs=1) as cpool,
            tc.tile_pool(name="xin", bufs=3) as xpool,
            tc.tile_pool(name="zst", bufs=3) as ztpool,
            tc.tile_pool(name="bsp", bufs=4) as bspool,
            tc.tile_pool(name="prods", bufs=6) as ppool,
            tc.tile_pool(name="outs", bufs=3) as opool,
            tc.tile_pool(name="ptr", bufs=1, space="PSUM") as ptr_pool,
            tc.tile_pool(name="pa", bufs=2, space="PSUM") as pa_pool,
            tc.tile_pool(name="pb", bufs=2, space="PSUM") as pb_pool,
            tc.tile_pool(name="po", bufs=1, space="PSUM") as po_pool,
        ):
            cws = cpool.tile([D, CONST_W], BF16)
            nc.sync.dma_start(cws[:], cw[:])
            p2s = cws[:, 0:KR]
            q2s = cws[:, KR : 2 * KR]
            wts = cws[:, 2 * KR : 2 * KR + K]
            sels = cws[:, 2 * KR + K : 2 * KR + K + SELW]
            ident = cws[:, 2 * KR + K + SELW : 2 * KR + K + SELW + D]
            identn = cws[:, 2 * KR + K + SELW + D :]

            x1v = x1.rearrange("(c a p) d -> c p a d", p=TILE, a=CHUNK_TILES)
            x2v = x2.rearrange("(c a p) d -> c p a d", p=TILE, a=CHUNK_TILES)

            prev = None

            def do_tail(zt, st, j):
                # PE: lin first (opens the outT accumulation group),
                # then per-chunk A/B matmuls with sel-reduce skewed behind
                outp = po_pool.tile([K, BLK], F32, name=f"outp{j}", tag="outp")
                nc.tensor.matmul(outp[:], wts, zt[:], start=True, stop=False)

                chunks = []  # (a_psum, prod_sb) pending sel-reduce

                def emit_sel(c, a_ps, b_sb):
                    prod = ppool.tile(
                        [128, BLK], BF16, name=f"prod{j}_{c}", tag="prod"
                    )
                    nc.vector.tensor_mul(prod[:], a_ps[:], b_sb[:])
                    nc.tensor.matmul(
                        outp[:],
                        sels[:, c * K : (c + 1) * K],
                        prod[:],
                        start=False,
                        stop=(c == NCHUNK - 1),
                    )

                for c in range(NCHUNK):
                    a = pa_pool.tile([128, BLK], F32, name=f"a{j}_{c}", tag="A")
                    nc.tensor.matmul(
                        a[:], p2s[:, ts(c, 128)], zt[:], start=True, stop=True
                    )
                    b = pb_pool.tile([128, BLK], F32, name=f"b{j}_{c}", tag="B")
                    nc.tensor.matmul(
                        b[:], q2s[:, ts(c, 128)], st[:], start=True, stop=True
                    )
                    bs = bspool.tile([128, BLK], BF16, name=f"bs{j}_{c}", tag="bs")
                    nc.scalar.copy(bs[:], b[:])
                    chunks.append((a, bs))
                    # skew: sel-reduce for chunk c-1 after chunk c's matmuls
                    if c >= 1:
                        emit_sel(c - 1, *chunks[c - 1])
                emit_sel(NCHUNK - 1, *chunks[NCHUNK - 1])

                # ACT: evacuate outT, then DMA [K, 512] f32 (2KB rows)
                osb = opool.tile([K, BLK], F32, name=f"osb{j}", tag="osb")
                nc.scalar.copy(osb[:], outp[:])
                nc.sync.dma_start(out[:, ts(j, BLK)], osb[:])

            for j in range(n_blocks):
                x1c = xpool.tile([TILE, CHUNK_TILES, D], BF16, name=f"x1c{j}", tag="x1c")
                nc.gpsimd.dma_start(x1c[:], x1v[j])
                x2c = xpool.tile([TILE, CHUNK_TILES, D], BF16, name=f"x2c{j}", tag="x2c")
                nc.gpsimd.dma_start(x2c[:], x2v[j])

                # PE: z^T/s^T via +/-I transposing matmuls, accumulated
                # in PSUM (pz bank0..1 region / ps bank..; per 128-col slice
                # the two groups hit different banks or run sequentially)
                pzs = ptr_pool.tile([D, 2 * BLK], F32, name=f"pzs{j}", tag="pzs")
                pz = pzs[:, 0:BLK]
                ps = pzs[:, BLK : 2 * BLK]
                for t in range(CHUNK_TILES):
                    x1b = x1c[:, t, :]
                    x2b = x2c[:, t, :]
                    pzt = pz[:, ts(t, TILE)]
                    pst = ps[:, ts(t, TILE)]
                    nc.tensor.matmul(pzt, x1b, ident, start=True, stop=False)
                    nc.tensor.matmul(pst, x1b, ident, start=True, stop=False)
                    nc.tensor.matmul(pzt, x2b, identn, start=False, stop=True)
                    nc.tensor.matmul(pst, x2b, ident, start=False, stop=True)

                # evacuate: zt on ACT, st on DVE (split to balance engines)
                zt = ztpool.tile([D, BLK], BF16, name=f"zt{j}", tag="zt")
                nc.scalar.copy(zt[:], pz)
                st = ztpool.tile([D, BLK], BF16, name=f"st{j}", tag="st")
                nc.vector.tensor_copy(st[:], ps)

                if prev is not None:
                    do_tail(*prev)
                prev = (zt, st, j)

            do_tail(*prev)

    nc.finalize()
    return nc


def _make_sel():
    sel = np.zeros((NCHUNK, 128, K), dtype=np.float32)
    for c in range(NCHUNK):
        for p in range(128):
            sel[c, p, c * 8 + p // 16] = 1.0
    return sel.transpose(1, 0, 2).reshape(128, NCHUNK * K)


def _shard_and_pack(x1, x2, W_lin, P, Q):
    p2 = P.transpose(1, 0, 2).reshape(D, KR)
    q2 = Q.transpose(1, 0, 2).reshape(D, KR)
    wt = np.ascontiguousarray(W_lin.T)
    idp = np.eye(D, dtype=np.float32)
    cwv = np.concatenate([p2, q2, wt, _make_sel(), idp, -idp], axis=1).astype(
        ml_dtypes.bfloat16
    )
    assert cwv.shape == (D, CONST_W)

    in_maps = []
    for b in range(N_CORES):
        in_maps.append(
            {
                "x1": np.ascontiguousarray(x1[b]),
                "x2": np.ascontiguousarray(x2[b]),
                "cw": cwv,
            }
        )
    return in_maps


def kernel(x1, x2, W_lin, P, Q):
    assert x1.shape == (N_CORES, 16384, D) and x2.shape == x1.shape
    nc = build_bass(16384)
    in_maps = _shard_and_pack(x1, x2, W_lin, P, Q)
    res = run_bass_kernel_spmd(nc, in_maps, core_ids=list(range(N_CORES)))
    out = np.stack(
        [np.ascontiguousarray(res.results[b]["out"].T) for b in range(N_CORES)],
        axis=0,
    )
    return out.astype(np.float32)
